# revision 54
# baseline (speedup 1.0000x reference)
"""Trainium2 Bass kernel for nn_Diagnet (S=1024, B=64, I=512, H=2048, O=512).

    u = einsum('sbi,hi->sbh', X, W_ih)
    h_t = |u_t + hh * h_{t-1}|   (scan over S, only final h needed)
    Y = h_final @ W_ho.T + b_ho

Strategy (8 NeuronCores, data-parallel over batch, 8 batch rows per core):

* H lanes are permuted so hh is sorted descending and split into 16
  chunks of 128.  With per-lane decay a = hh < 1, a chunk whose largest
  a satisfies a^K < 1e-10 only needs the last K steps (exact to ~1e-10,
  far below fp32 noise): the GEMM and the scan skip everything earlier.
* The scan runs as ONE custom DVE instruction per (chunk, 64-step
  block): a hand-tuned micro-op program (SCANDIAG_ANT) folds
  m_k = |m_{k-1} - x_k * sc_k| along the free dimension at 1 elem/cycle,
  with the 8 batch chains as sub-dimension pages that re-init the fold
  state to 0 at each page boundary.  State is kept pre-scaled
  (m = a^(K-1-t) h, scales folded into the sc stream), so the step is
  multiply-free in the recurrence itself and no inter-block rescale is
  needed.  Block-to-block state is chained by a carry element prepended
  to each page, written by the previous block's scan through a stride-0
  output AP (65 writes to one address; last write = final state).  The
  scan outputs -m so the carry re-seeds exactly: |0 - (-m)| = m.
* GEMM in bf16 (1 PE cycle/row vs 4 for fp32): X and W_ih^T are cast
  host-side.  PSUM drains to SBUF as plain copies on the otherwise-idle
  Activation engine; the scale (and its sign flip) ride the scan's
  second input stream.  A chunk's first active block has no carry, so
  its scan reads PSUM directly (no copy, no memset) and covers only the
  chunk's true truncation tail (unrounded window).
* Scheduling: chunks whose whole window is the last block have zero
  dependency on the rest of the scan; they run as fillers during the
  chunk-0-only phase where the engines idle on X DMAs.  X streams in
  2-block-pair DMAs (DMA-completion semaphores cost ~2us each), and
  large constants are issued at points chosen to keep them out of the
  phase-1 HBM burst.
* Final projection: h arrives in [128, (chunk, batch)] layout; 16
  accumulating bf16 matmuls against host-transposed-negated W_ho^T
  (negated because the scan hands over -h), bias added on DVE.  Filler
  chunks' matmuls run early, off the tail.
"""

import math
import os

from contextlib import ExitStack

import numpy as np

S, B, I, H, O = 1024, 64, 512, 2048, 512
NCORES = 8
BC = B // NCORES  # 8 batch rows per core
TB = 64  # time block
NBLK = S // TB  # 16
NCH = H // 128  # 16 h-chunks
NI = I // 128  # 4 i-chunks
NSLOT = TB + 1  # carry + 64 scan elements per page
LN_TRUNC = 23.03  # a^K <= e^-23 ~ 1e-10 -> truncate

_CACHE = {}


def _register_scandiag():
    """Custom DVE op: per page s of [P, S, N] in0 (in1 broadcast over pages):
        m = 0
        for k: m = |m - in0[p,s,k] * in1[p,k]| ; out[p,s,k] = -m
    Built from lower() of a Spec, with the step uop hand-edited so the
    fold state re-inits to 0 at each page boundary."""
    import concourse.dve_ops as dve_ops
    from concourse.dve_spec import Spec, Src0, Src1, Zero, scan, PageIdx, lower, AluOp
    from concourse.dve_uop import DveOpSpec, AluInp

    name = "SCANDIAG_ANT"
    for op in dve_ops.OPS:
        if op.name == name:
            return op

    body = (Zero - scan(AluOp.ABSOLUTE_DIFF, Src0 * Src1, init=Zero)) + PageIdx(
        Zero, Zero
    )

    def ref(in0, in1, s0, s1, imm2):
        x = in0.astype(np.float32)
        w = in1.astype(np.float32)
        if x.ndim == 2:
            x = x[:, None, :]
        if w.ndim == 3:
            w = w[:, 0, :]
        out = np.empty_like(x)
        for s in range(x.shape[1]):
            m = np.zeros(x.shape[0], dtype=np.float32)
            for k in range(x.shape[2]):
                m = np.abs(m - x[:, s, k] * w[:, k])
                out[:, s, k] = -m
        return out.reshape(in0.shape)

    spec = Spec(body=body, reference=ref)
    row = max(dve_ops._SUB_OPCODE_FOR_NAME.values()) + 1
    assert row < 0x20
    shas = {}
    compiled = {}
    for ver in ("v3", "v4"):
        uops = lower(spec, ver=ver)
        assert len(uops) == 3, (ver, len(uops))
        hits = [
            k
            for k, b in enumerate(uops[2].datapath_config)
            if b.op == AluOp.ABSOLUTE_DIFF and b.alu_src0 == AluInp.CURR_ALU_OUT
        ]
        assert len(hits) == 1, hits
        k = hits[0]
        seed_blk = uops[0].datapath_config[k]
        assert seed_blk.op == AluOp.BYPASS
        uops[2].datapath_config[k].alu_src0 = seed_blk.alu_src0
        s = DveOpSpec(name=name, opcode=row, uops=uops, rd1_en=True)
        shas[ver] = s.sha(ver)
        compiled[ver] = s
    op = dve_ops.DveOp(name, spec, subdim=True, uops_sha=shas)
    for ver in ("v3", "v4"):
        dve_ops._COMPILE_CACHE[(name, ver)] = compiled[ver]
    dve_ops._SUB_OPCODE_FOR_NAME[name] = row
    dve_ops.OPS.append(op)
    dve_ops.CUSTOM_DVE_SPECS[name] = spec
    return op


def _make_plan(hh):
    a = np.maximum(np.abs(hh.astype(np.float64)), 1e-30)
    perm = np.argsort(-a, kind="stable")
    a_s = a[perm]
    first_block = []
    fwidth = []  # steps computed in the chunk's first block (1..TB)
    for g in range(NCH):
        amax = a_s[g * 128]
        if amax >= math.exp(-LN_TRUNC / S):
            kg = S
        else:
            kg = min(S, max(1, int(math.ceil(LN_TRUNC / math.log(1.0 / amax)))))
        nblk_g = (kg + TB - 1) // TB
        first_block.append(NBLK - nblk_g)
        fwidth.append(kg - (nblk_g - 1) * TB)
    assert all(
        first_block[g] <= first_block[g + 1] for g in range(NCH - 1)
    ), first_block

    # SC stream: kb-major variable-width segments. A chunk's first block
    # covers only its true window tail (width fwidth[g], no carry slot);
    # later blocks are 65 wide: carry slot (1.0) + 64 scales.
    # scale slot for step tau-from-window-start: -a^(K-1-tau).
    ag = a_s.reshape(NCH, 128).T  # [128, NCH]
    segs = [(g, kb) for kb in range(NBLK) for g in range(NCH) if first_block[g] <= kb]
    seg_off = {}
    widths = {}
    off = 0
    for g, kb in segs:
        w = fwidth[g] if kb == first_block[g] else NSLOT
        seg_off[(g, kb)] = off
        widths[(g, kb)] = w
        off += w
    sc = np.zeros((128, off), dtype=np.float64)
    for (g, kb), o in seg_off.items():
        kg = fwidth[g] + (NBLK - 1 - first_block[g]) * TB
        if kb == first_block[g]:
            w = fwidth[g]
            tau = np.arange(w)
            e = kg - 1 - tau
            sc[:, o : o + w] = -(ag[:, g : g + 1] ** e[None, :])
        else:
            tau = np.arange(TB)
            e = kg - 1 - (fwidth[g] + (kb - 1 - first_block[g]) * TB + tau)
            sc[:, o] = 1.0
            sc[:, o + 1 : o + NSLOT] = -(ag[:, g : g + 1] ** e[None, :])
    nearly = sum(widths[s] for s in segs if s[1] < first_block[1])
    return {
        "perm": perm,
        "first_block": tuple(first_block),
        "fwidth": tuple(fwidth),
        "SC": sc,  # float64; cast at upload
        "sclen": off,
        "nearly": max(nearly, 1),
        "seg_off": seg_off,
    }


def _build(first_block, fwidth, sclen, nearly, seg_off):
    import concourse.mybir as mybir
    import concourse.tile as tile
    from concourse import bacc
    from concourse.bass import ds

    SCANDIAG = _register_scandiag()
    f32 = mybir.dt.float32
    bf16 = mybir.dt.bfloat16

    nc = bacc.Bacc("TRN2", target_bir_lowering=False, debug=False, num_devices=NCORES)
    NPAIR = NBLK // 2
    BLKF = NI * BC * TB  # free elems per block
    X = nc.dram_tensor("X", [NPAIR, 128, 2 * BLKF], bf16, kind="ExternalInput").ap()
    WIHT = nc.dram_tensor("WIHT", [I, H], bf16, kind="ExternalInput").ap()
    SC = nc.dram_tensor("SC", [128, sclen], bf16, kind="ExternalInput").ap()
    WHOT = nc.dram_tensor("WHOT", [H, O], bf16, kind="ExternalInput").ap()
    BIAS = nc.dram_tensor("BIAS", [BC, O], f32, kind="ExternalInput").ap()
    Y = nc.dram_tensor("Y", [BC, O], f32, kind="ExternalOutput").ap()

    acts = [[g for g in range(NCH) if first_block[g] <= kb] for kb in range(NBLK)]

    with tile.TileContext(nc) as tc:
        with ExitStack() as ctx:
            consts = ctx.enter_context(tc.tile_pool(name="consts", bufs=1))
            xtpool = ctx.enter_context(
                tc.tile_pool(name="xt", bufs=int(os.environ.get("DIAG_XB", "3")))
            )
            upool = ctx.enter_context(tc.tile_pool(name="ubuf", bufs=3))
            ypool = ctx.enter_context(tc.tile_pool(name="yout", bufs=1))
            gpool = ctx.enter_context(
                tc.tile_pool(name="gpsum", bufs=int(os.environ.get("DIAG_GP", "7")), space="PSUM")
            )
            fpool = ctx.enter_context(tc.tile_pool(name="fpsum", bufs=1, space="PSUM"))

            # --- constants ---
            # chunk-0 columns as a small separate tile: blocks 0..fb[1]-1 only
            # touch chunk 0, and this 128KB DMA unblocks the first GEMMs fast.
            wih0 = consts.tile([128, NI * 128], bf16, tag="wih0", name="wih0")
            nc.sync.dma_start(
                wih0[:].rearrange("p (i h) -> p i h", i=NI),
                WIHT.rearrange("(i p) h -> p i h", p=128)[:, :, 0:128],
            )
            sc_t = consts.tile([128, sclen], bf16, tag="sc", name="sc_t")
            ne = nearly
            nc.sync.dma_start(sc_t[:, 0:ne], SC[:, 0:ne])
            wiht = consts.tile([128, NI * H], bf16, tag="wiht", name="wiht")
            w4 = wiht[:].rearrange("p (i h) -> p i h", i=NI)
            hbuf = consts.tile([128, NCH * BC], bf16, tag="hbuf", name="hbuf")

            ucur = {}

            def wslice(g, ic):
                if g == 0:
                    return wih0[:].rearrange("p (i h) -> p i h", i=NI)[:, ic]
                return w4[:, ic, ds(g * 128, 128)]

            def process_seg(g, kb, x3):
                first = kb == first_block[g]
                last = kb == NBLK - 1
                off = seg_off[(g, kb)]
                w = fwidth[g] if first else TB
                ps = gpool.tile([128, w * BC], f32, tag="gp", name=f"gp_{kb}_{g}")
                ps3 = ps[:].rearrange("p (s n) -> p s n", s=BC)
                x4 = x3.rearrange("p i (s t) -> p i s t", s=BC)
                for ic in range(NI):
                    nc.tensor.matmul(
                        ps3,
                        wslice(g, ic),
                        x4[:, ic, :, TB - w : TB],
                        start=(ic == 0),
                        stop=(ic == NI - 1),
                    )
                if first:
                    # no carry yet: scan straight out of PSUM, skip the copy;
                    # only the chunk's true window tail (w steps) is computed
                    in0 = ps3
                    width = w
                    scs = sc_t[:, ds(off, w)].unsqueeze(1).broadcast_to(
                        [128, BC, w]
                    )
                else:
                    u3 = ucur[g][:].rearrange("p (s n) -> p s n", n=NSLOT)
                    nc.scalar.copy(u3[:, :, 1:NSLOT], ps3)
                    in0 = u3
                    width = NSLOT
                    scs = sc_t[:, ds(off, NSLOT)].unsqueeze(1).broadcast_to(
                        [128, BC, NSLOT]
                    )
                if last:
                    out_ap = hbuf[:, ds(g * BC, BC)].broadcast_to([128, BC, width])
                else:
                    u_nxt = upool.tile(
                        [128, BC * NSLOT], f32, tag=f"u{g}", name=f"u_{g}_{kb + 1}"
                    )
                    out_ap = (
                        u_nxt[:]
                        .rearrange("p (s n) -> p s n", n=NSLOT)[:, :, 0:1]
                        .broadcast_to([128, BC, width])
                    )
                nc.vector._custom_dve(SCANDIAG, out=out_ap, in0=in0, in1=scs)
                if not last:
                    ucur[g] = u_nxt

            # chunks whose whole window is block 15: zero deps on the chain.
            # Run them as fillers during the chunk-0-only phase, where DVE/Act
            # and the PE are mostly idle waiting on X DMAs.
            if int(os.environ.get("DIAG_FILL", "1")):
                fillers = [g for g in range(1, NCH) if first_block[g] == NBLK - 1]
            else:
                fillers = []
            FILL_AT = list(range(8, 14))
            fill_sched = {kb: [] for kb in FILL_AT}
            for i, g in enumerate(fillers):
                fill_sched[FILL_AT[i % len(FILL_AT)]].append(g)

            x15 = consts.tile([128, 2 * BLKF], bf16, tag="x15", name="x15")
            x15v = x15[:].rearrange("p (two i n) -> p two i n", two=2, i=NI)

            xpair = None
            for kb in range(NBLK):
                # X: one DMA per 2-block pair (each DMA completion semaphore
                # costs ~2us, so fewer/larger transfers win); block 0 gets its
                # own half-DMA so the pipeline starts sooner; the last pair
                # lives in a dedicated tile loaded early for the fillers.
                if kb % 2 == 0 and kb < NBLK - 2:
                    xpair = xtpool.tile(
                        [128, 2 * BLKF], bf16, tag="xt", name=f"xt_{kb}"
                    )
                    if kb == 0:
                        nc.gpsimd.dma_start(xpair[:, 0:BLKF], X[0][:, 0:BLKF])
                        nc.gpsimd.dma_start(xpair[:, BLKF : 2 * BLKF], X[0][:, BLKF:])
                    else:
                        nc.gpsimd.dma_start(xpair[:], X[kb // 2])
                if kb < NBLK - 2:
                    x3 = xpair[:].rearrange(
                        "p (two i n) -> p two i n", two=2, i=NI
                    )[:, kb % 2]
                else:
                    x3 = x15v[:, kb % 2]

                if kb == 2:
                    # rest of the scale table (not needed until block fb[1])
                    nc.sync.dma_start(sc_t[:, ne:], SC[:, ne:])
                if kb == 3:
                    # last block pair, needed by fillers from kb~8
                    nc.gpsimd.dma_start(x15[:], X[NPAIR - 1])
                if kb == 5:
                    # full W_ih columns: first needed by the fillers
                    nc.sync.dma_start(w4, WIHT.rearrange("(i p) h -> p i h", p=128))
                if kb == 8:
                    # needed only at the end; issued here to overlap
                    whot = consts.tile([128, NCH * O], bf16, tag="whot", name="whot")
                    nc.sync.dma_start(
                        whot[:].rearrange("p (g o) -> p g o", g=NCH),
                        WHOT.rearrange("(g p) o -> p g o", p=128),
                    )
                    bias_t = ypool.tile([BC, O], f32, tag="bias", name="bias_t")
                    nc.sync.dma_start(bias_t[:], BIAS)

                for g in acts[kb]:
                    if kb == NBLK - 1 and g in fillers:
                        continue  # already done as a filler
                    process_seg(g, kb, x3)
                for g in fill_sched.get(kb, ()):
                    process_seg(g, NBLK - 1, x15v[:, 1])

                if kb == 13 and fillers:
                    # final-projection matmuls for the filler chunks: their h
                    # is final and W_ho has landed, so take them off the tail
                    psy = fpool.tile([BC, O], f32, tag="fy", name="psy")
                    wh3 = whot[:].rearrange("p (g o) -> p g o", g=NCH)
                    for i, g in enumerate(fillers):
                        nc.tensor.matmul(
                            psy[:],
                            hbuf[:, ds(g * BC, BC)],
                            wh3[:, g],
                            start=(i == 0),
                            stop=False,
                        )

            # --- final projection: Y = (-h)^T @ (-W_ho^T) + bias ---
            if not fillers:
                psy = fpool.tile([BC, O], f32, tag="fy", name="psy")
                wh3 = whot[:].rearrange("p (g o) -> p g o", g=NCH)
            tail = [g for g in range(NCH) if g not in fillers]
            for i, g in enumerate(tail):
                nc.tensor.matmul(
                    psy[:],
                    hbuf[:, ds(g * BC, BC)],
                    wh3[:, g],
                    start=(not fillers) and i == 0,
                    stop=(i == len(tail) - 1),
                )
            y_t = ypool.tile([BC, O], f32, tag="y", name="y_t")
            nc.vector.tensor_tensor(y_t[:], psy[:], bias_t[:], mybir.AluOpType.add)
            nc.sync.dma_start(Y, y_t[:])
    nc.compile()
    return nc


def _get_program(plan):
    key = (plan["first_block"], plan["fwidth"], os.environ.get("DIAG_GP"))
    if key not in _CACHE:
        _CACHE[key] = _build(
            plan["first_block"],
            plan["fwidth"],
            plan["sclen"],
            plan["nearly"],
            plan["seg_off"],
        )
    return _CACHE[key]


def _ensure_ntff_hook():
    """Provide antenv.axon_hooks (absent in this image) so trace=True works."""
    import sys
    import types

    if "antenv.axon_hooks" in sys.modules:
        return True
    try:
        import antenv

        mod = types.ModuleType("antenv.axon_hooks")
        mod._hook = None

        def set_axon_ntff_profile_hook(h):
            mod._hook = h

        def get_axon_ntff_profile_hook():
            return mod._hook

        mod.set_axon_ntff_profile_hook = set_axon_ntff_profile_hook
        mod.get_axon_ntff_profile_hook = get_axon_ntff_profile_hook
        sys.modules["antenv.axon_hooks"] = mod
        antenv.axon_hooks = mod

        from trn_agent_boot.trn_boot import _ntff_profile_via_ctypes

        hook = _ntff_profile_via_ctypes("/opt/axon/libaxon_pjrt.so")
        mod.set_axon_ntff_profile_hook(hook)
        return hook is not None
    except Exception:
        return False


def kernel(X, W_ih, hh, W_ho, b_ho):
    import ml_dtypes
    from concourse import bass_utils

    bf16 = ml_dtypes.bfloat16
    X = np.asarray(X, dtype=np.float32)
    W_ih = np.asarray(W_ih, dtype=np.float32)
    hh = np.asarray(hh, dtype=np.float32)
    W_ho = np.asarray(W_ho, dtype=np.float32)
    b_ho = np.asarray(b_ho, dtype=np.float32)

    plan = _make_plan(hh)
    perm = plan["perm"]
    nc = _get_program(plan)

    wiht = np.ascontiguousarray(W_ih[perm].T).astype(bf16)  # [I, H]
    whot = np.ascontiguousarray(-W_ho[:, perm].T).astype(bf16)  # [H, O], negated
    bias = np.tile(b_ho[None, :], (BC, 1)).astype(np.float32)

    common = {
        "WIHT": wiht,
        "WHOT": whot,
        "BIAS": bias,
        "SC": plan["SC"].astype(bf16),
    }
    in_maps = []
    for m in range(NCORES):
        im = dict(common)
        xm = X[:, m * BC : (m + 1) * BC, :]  # [S, BC, I]
        # device layout [NBLK/2, 128(i-within), (block-in-pair, ic, b, tau)]
        xt = xm.reshape(NBLK // 2, 2, TB, BC, NI, 128).transpose(0, 5, 1, 4, 3, 2)
        im["X"] = (
            np.ascontiguousarray(xt)
            .reshape(NBLK // 2, 128, 2 * NI * BC * TB)
            .astype(bf16)
        )
        in_maps.append(im)

    trace = bool(int(os.environ.get("DIAG_TRACE", "0")))
    if trace:
        trace = _ensure_ntff_hook()
    res = None
    for attempt in range(3):
        try:
            res = bass_utils.run_bass_kernel_spmd(
                nc,
                in_maps,
                core_ids=list(range(NCORES)),
                trace=trace,
                tmpdir=os.environ.get("DIAG_TRACE_DIR") or None,
            )
            break
        except Exception:
            if attempt == 2:
                raise
            trace = False  # retry without profiling
    if res.exec_time_ns is not None:
        kernel.last_exec_time_ns = res.exec_time_ns
        kernel.last_mean_exec_time_ns = res.mean_exec_time_ns
    Yfull = np.concatenate([r["Y"] for r in res.results], axis=0)
    return Yfull


kernel.last_exec_time_ns = None
kernel.last_mean_exec_time_ns = None


# revision 55
# speedup vs baseline: 1.0156x; 1.0156x over previous
"""Trainium2 Bass kernel for nn_Diagnet (S=1024, B=64, I=512, H=2048, O=512).

    u = einsum('sbi,hi->sbh', X, W_ih)
    h_t = |u_t + hh * h_{t-1}|   (scan over S, only final h needed)
    Y = h_final @ W_ho.T + b_ho

Strategy (8 NeuronCores, data-parallel over batch, 8 batch rows per core):

* H lanes are permuted so hh is sorted descending and split into 16
  chunks of 128.  With per-lane decay a = hh < 1, a chunk whose largest
  a satisfies a^K < 1e-10 only needs the last K steps (exact to ~1e-10,
  far below fp32 noise): the GEMM and the scan skip everything earlier.
* The scan runs as ONE custom DVE instruction per (chunk, 64-step
  block): a hand-tuned micro-op program (SCANDIAG_ANT) folds
  m_k = |m_{k-1} - x_k * sc_k| along the free dimension at 1 elem/cycle,
  with the 8 batch chains as sub-dimension pages that re-init the fold
  state to 0 at each page boundary.  State is kept pre-scaled
  (m = a^(K-1-t) h, scales folded into the sc stream), so the step is
  multiply-free in the recurrence itself and no inter-block rescale is
  needed.  Block-to-block state is chained by a carry element prepended
  to each page, written by the previous block's scan through a stride-0
  output AP (65 writes to one address; last write = final state).  The
  scan outputs -m so the carry re-seeds exactly: |0 - (-m)| = m.
* GEMM in bf16 (1 PE cycle/row vs 4 for fp32): X and W_ih^T are cast
  host-side.  PSUM drains to SBUF as plain copies on the otherwise-idle
  Activation engine; the scale (and its sign flip) ride the scan's
  second input stream.  A chunk's first active block has no carry, so
  its scan reads PSUM directly (no copy, no memset) and covers only the
  chunk's true truncation tail (unrounded window).
* Scheduling: chunks whose whole window is the last block have zero
  dependency on the rest of the scan; they run as fillers during the
  chunk-0-only phase where the engines idle on X DMAs.  X streams in
  2-block-pair DMAs (DMA-completion semaphores cost ~2us each), and
  large constants are issued at points chosen to keep them out of the
  phase-1 HBM burst.
* Final projection: h arrives in [128, (chunk, batch)] layout; 16
  accumulating bf16 matmuls against host-transposed-negated W_ho^T
  (negated because the scan hands over -h), bias added on DVE.  Filler
  chunks' matmuls run early, off the tail.
"""

import math
import os

from contextlib import ExitStack

import numpy as np

S, B, I, H, O = 1024, 64, 512, 2048, 512
NCORES = 8
BC = B // NCORES  # 8 batch rows per core
TB = 64  # time block
NBLK = S // TB  # 16
NCH = H // 128  # 16 h-chunks
NI = I // 128  # 4 i-chunks
NSLOT = TB + 1  # carry + 64 scan elements per page
LN_TRUNC = 23.03  # a^K <= e^-23 ~ 1e-10 -> truncate

_CACHE = {}


def _register_scandiag():
    """Custom DVE op: per page s of [P, S, N] in0 (in1 broadcast over pages):
        m = 0
        for k: m = |m - in0[p,s,k] * in1[p,k]| ; out[p,s,k] = -m
    Built from lower() of a Spec, with the step uop hand-edited so the
    fold state re-inits to 0 at each page boundary."""
    import concourse.dve_ops as dve_ops
    from concourse.dve_spec import Spec, Src0, Src1, Zero, scan, PageIdx, lower, AluOp
    from concourse.dve_uop import DveOpSpec, AluInp

    name = "SCANDIAG_ANT"
    for op in dve_ops.OPS:
        if op.name == name:
            return op

    body = (Zero - scan(AluOp.ABSOLUTE_DIFF, Src0 * Src1, init=Zero)) + PageIdx(
        Zero, Zero
    )

    def ref(in0, in1, s0, s1, imm2):
        x = in0.astype(np.float32)
        w = in1.astype(np.float32)
        if x.ndim == 2:
            x = x[:, None, :]
        if w.ndim == 3:
            w = w[:, 0, :]
        out = np.empty_like(x)
        for s in range(x.shape[1]):
            m = np.zeros(x.shape[0], dtype=np.float32)
            for k in range(x.shape[2]):
                m = np.abs(m - x[:, s, k] * w[:, k])
                out[:, s, k] = -m
        return out.reshape(in0.shape)

    spec = Spec(body=body, reference=ref)
    row = max(dve_ops._SUB_OPCODE_FOR_NAME.values()) + 1
    assert row < 0x20
    shas = {}
    compiled = {}
    for ver in ("v3", "v4"):
        uops = lower(spec, ver=ver)
        assert len(uops) == 3, (ver, len(uops))
        hits = [
            k
            for k, b in enumerate(uops[2].datapath_config)
            if b.op == AluOp.ABSOLUTE_DIFF and b.alu_src0 == AluInp.CURR_ALU_OUT
        ]
        assert len(hits) == 1, hits
        k = hits[0]
        seed_blk = uops[0].datapath_config[k]
        assert seed_blk.op == AluOp.BYPASS
        uops[2].datapath_config[k].alu_src0 = seed_blk.alu_src0
        s = DveOpSpec(name=name, opcode=row, uops=uops, rd1_en=True)
        shas[ver] = s.sha(ver)
        compiled[ver] = s
    op = dve_ops.DveOp(name, spec, subdim=True, uops_sha=shas)
    for ver in ("v3", "v4"):
        dve_ops._COMPILE_CACHE[(name, ver)] = compiled[ver]
    dve_ops._SUB_OPCODE_FOR_NAME[name] = row
    dve_ops.OPS.append(op)
    dve_ops.CUSTOM_DVE_SPECS[name] = spec
    return op


def _make_plan(hh):
    a = np.maximum(np.abs(hh.astype(np.float64)), 1e-30)
    perm = np.argsort(-a, kind="stable")
    a_s = a[perm]
    first_block = []
    fwidth = []  # steps computed in the chunk's first block (1..TB)
    for g in range(NCH):
        amax = a_s[g * 128]
        if amax >= math.exp(-LN_TRUNC / S):
            kg = S
        else:
            kg = min(S, max(1, int(math.ceil(LN_TRUNC / math.log(1.0 / amax)))))
        nblk_g = (kg + TB - 1) // TB
        first_block.append(NBLK - nblk_g)
        fwidth.append(kg - (nblk_g - 1) * TB)
    assert all(
        first_block[g] <= first_block[g + 1] for g in range(NCH - 1)
    ), first_block

    # SC stream: kb-major variable-width segments. A chunk's first block
    # covers only its true window tail (width fwidth[g], no carry slot);
    # later blocks are 65 wide: carry slot (1.0) + 64 scales.
    # scale slot for step tau-from-window-start: -a^(K-1-tau).
    ag = a_s.reshape(NCH, 128).T  # [128, NCH]
    segs = [(g, kb) for kb in range(NBLK) for g in range(NCH) if first_block[g] <= kb]
    seg_off = {}
    widths = {}
    off = 0
    for g, kb in segs:
        w = fwidth[g] if kb == first_block[g] else NSLOT
        seg_off[(g, kb)] = off
        widths[(g, kb)] = w
        off += w
    sc = np.zeros((128, off), dtype=np.float64)
    for (g, kb), o in seg_off.items():
        kg = fwidth[g] + (NBLK - 1 - first_block[g]) * TB
        if kb == first_block[g]:
            w = fwidth[g]
            tau = np.arange(w)
            e = kg - 1 - tau
            sc[:, o : o + w] = -(ag[:, g : g + 1] ** e[None, :])
        else:
            tau = np.arange(TB)
            e = kg - 1 - (fwidth[g] + (kb - 1 - first_block[g]) * TB + tau)
            sc[:, o] = 1.0
            sc[:, o + 1 : o + NSLOT] = -(ag[:, g : g + 1] ** e[None, :])
    nearly = sum(widths[s] for s in segs if s[1] < first_block[1])
    return {
        "perm": perm,
        "first_block": tuple(first_block),
        "fwidth": tuple(fwidth),
        "SC": sc,  # float64; cast at upload
        "sclen": off,
        "nearly": max(nearly, 1),
        "seg_off": seg_off,
    }


def _build(first_block, fwidth, sclen, nearly, seg_off):
    import concourse.mybir as mybir
    import concourse.tile as tile
    from concourse import bacc
    from concourse.bass import ds

    SCANDIAG = _register_scandiag()
    f32 = mybir.dt.float32
    bf16 = mybir.dt.bfloat16

    nc = bacc.Bacc("TRN2", target_bir_lowering=False, debug=False, num_devices=NCORES)
    NPAIR = NBLK // 2
    BLKF = NI * BC * TB  # free elems per block
    X = nc.dram_tensor("X", [NPAIR, 128, 2 * BLKF], bf16, kind="ExternalInput").ap()
    WIHT = nc.dram_tensor("WIHT", [I, H], bf16, kind="ExternalInput").ap()
    SC = nc.dram_tensor("SC", [128, sclen], bf16, kind="ExternalInput").ap()
    WHOT = nc.dram_tensor("WHOT", [H, O], bf16, kind="ExternalInput").ap()
    BIAS = nc.dram_tensor("BIAS", [BC, O], f32, kind="ExternalInput").ap()
    Y = nc.dram_tensor("Y", [BC, O], f32, kind="ExternalOutput").ap()

    acts = [[g for g in range(NCH) if first_block[g] <= kb] for kb in range(NBLK)]

    with tile.TileContext(nc) as tc:
        with ExitStack() as ctx:
            consts = ctx.enter_context(tc.tile_pool(name="consts", bufs=1))
            xtpool = ctx.enter_context(
                tc.tile_pool(name="xt", bufs=int(os.environ.get("DIAG_XB", "3")))
            )
            upool = ctx.enter_context(tc.tile_pool(name="ubuf", bufs=int(os.environ.get("DIAG_UB", "3"))))
            ypool = ctx.enter_context(tc.tile_pool(name="yout", bufs=1))
            gpool = ctx.enter_context(
                tc.tile_pool(name="gpsum", bufs=int(os.environ.get("DIAG_GP", "7")), space="PSUM")
            )
            fpool = ctx.enter_context(tc.tile_pool(name="fpsum", bufs=1, space="PSUM"))

            # --- constants ---
            # chunk-0 columns as a small separate tile: blocks 0..fb[1]-1 only
            # touch chunk 0, and this 128KB DMA unblocks the first GEMMs fast.
            wih0 = consts.tile([128, NI * 128], bf16, tag="wih0", name="wih0")
            nc.sync.dma_start(
                wih0[:].rearrange("p (i h) -> p i h", i=NI),
                WIHT.rearrange("(i p) h -> p i h", p=128)[:, :, 0:128],
            )
            sc_t = consts.tile([128, sclen], bf16, tag="sc", name="sc_t")
            ne = nearly
            nc.sync.dma_start(sc_t[:, 0:ne], SC[:, 0:ne])
            wiht = consts.tile([128, NI * H], bf16, tag="wiht", name="wiht")
            w4 = wiht[:].rearrange("p (i h) -> p i h", i=NI)
            hbuf = consts.tile([128, NCH * BC], bf16, tag="hbuf", name="hbuf")

            ucur = {}

            def wslice(g, ic):
                if g == 0:
                    return wih0[:].rearrange("p (i h) -> p i h", i=NI)[:, ic]
                return w4[:, ic, ds(g * 128, 128)]

            def process_seg(g, kb, x3):
                first = kb == first_block[g]
                last = kb == NBLK - 1
                off = seg_off[(g, kb)]
                w = fwidth[g] if first else TB
                ps = gpool.tile([128, w * BC], f32, tag="gp", name=f"gp_{kb}_{g}")
                ps3 = ps[:].rearrange("p (s n) -> p s n", s=BC)
                x4 = x3.rearrange("p i (s t) -> p i s t", s=BC)
                for ic in range(NI):
                    nc.tensor.matmul(
                        ps3,
                        wslice(g, ic),
                        x4[:, ic, :, TB - w : TB],
                        start=(ic == 0),
                        stop=(ic == NI - 1),
                    )
                if first:
                    # no carry yet: scan straight out of PSUM, skip the copy;
                    # only the chunk's true window tail (w steps) is computed
                    in0 = ps3
                    width = w
                    scs = sc_t[:, ds(off, w)].unsqueeze(1).broadcast_to(
                        [128, BC, w]
                    )
                else:
                    u3 = ucur[g][:].rearrange("p (s n) -> p s n", n=NSLOT)
                    nc.scalar.copy(u3[:, :, 1:NSLOT], ps3)
                    in0 = u3
                    width = NSLOT
                    scs = sc_t[:, ds(off, NSLOT)].unsqueeze(1).broadcast_to(
                        [128, BC, NSLOT]
                    )
                if last:
                    out_ap = hbuf[:, ds(g * BC, BC)].broadcast_to([128, BC, width])
                else:
                    u_nxt = upool.tile(
                        [128, BC * NSLOT], f32, tag=f"u{g}", name=f"u_{g}_{kb + 1}"
                    )
                    out_ap = (
                        u_nxt[:]
                        .rearrange("p (s n) -> p s n", n=NSLOT)[:, :, 0:1]
                        .broadcast_to([128, BC, width])
                    )
                nc.vector._custom_dve(SCANDIAG, out=out_ap, in0=in0, in1=scs)
                if not last:
                    ucur[g] = u_nxt

            # chunks whose whole window is block 15: zero deps on the chain.
            # Run them as fillers during the chunk-0-only phase, where DVE/Act
            # and the PE are mostly idle waiting on X DMAs.
            if int(os.environ.get("DIAG_FILL", "1")):
                fillers = [g for g in range(1, NCH) if first_block[g] == NBLK - 1]
            else:
                fillers = []
            FILL_AT = list(range(8, 14))
            fill_sched = {kb: [] for kb in FILL_AT}
            for i, g in enumerate(fillers):
                fill_sched[FILL_AT[i % len(FILL_AT)]].append(g)

            x15 = consts.tile([128, 2 * BLKF], bf16, tag="x15", name="x15")
            x15v = x15[:].rearrange("p (two i n) -> p two i n", two=2, i=NI)

            xpair = None
            for kb in range(NBLK):
                # X: one DMA per 2-block pair (each DMA completion semaphore
                # costs ~2us, so fewer/larger transfers win); block 0 gets its
                # own half-DMA so the pipeline starts sooner; the last pair
                # lives in a dedicated tile loaded early for the fillers.
                if kb % 2 == 0 and kb < NBLK - 2:
                    xpair = xtpool.tile(
                        [128, 2 * BLKF], bf16, tag="xt", name=f"xt_{kb}"
                    )
                    if kb == 0:
                        nc.gpsimd.dma_start(xpair[:, 0:BLKF], X[0][:, 0:BLKF])
                        nc.gpsimd.dma_start(xpair[:, BLKF : 2 * BLKF], X[0][:, BLKF:])
                    else:
                        nc.gpsimd.dma_start(xpair[:], X[kb // 2])
                if kb < NBLK - 2:
                    x3 = xpair[:].rearrange(
                        "p (two i n) -> p two i n", two=2, i=NI
                    )[:, kb % 2]
                else:
                    x3 = x15v[:, kb % 2]

                if kb == 2:
                    # rest of the scale table (not needed until block fb[1])
                    nc.sync.dma_start(sc_t[:, ne:], SC[:, ne:])
                if kb == 3:
                    # last block pair, needed by fillers from kb~8
                    nc.gpsimd.dma_start(x15[:], X[NPAIR - 1])
                if kb == 5:
                    # full W_ih columns: first needed by the fillers
                    nc.sync.dma_start(w4, WIHT.rearrange("(i p) h -> p i h", p=128))
                if kb == 8:
                    # needed only at the end; issued here to overlap
                    whot = consts.tile([128, NCH * O], bf16, tag="whot", name="whot")
                    nc.sync.dma_start(
                        whot[:].rearrange("p (g o) -> p g o", g=NCH),
                        WHOT.rearrange("(g p) o -> p g o", p=128),
                    )
                    bias_t = ypool.tile([BC, O], f32, tag="bias", name="bias_t")
                    nc.sync.dma_start(bias_t[:], BIAS)

                for g in acts[kb]:
                    if kb == NBLK - 1 and g in fillers:
                        continue  # already done as a filler
                    process_seg(g, kb, x3)
                for g in fill_sched.get(kb, ()):
                    process_seg(g, NBLK - 1, x15v[:, 1])

                if kb == 13 and fillers:
                    # final-projection matmuls for the filler chunks: their h
                    # is final and W_ho has landed, so take them off the tail
                    psy = fpool.tile([BC, O], f32, tag="fy", name="psy")
                    wh3 = whot[:].rearrange("p (g o) -> p g o", g=NCH)
                    for i, g in enumerate(fillers):
                        nc.tensor.matmul(
                            psy[:],
                            hbuf[:, ds(g * BC, BC)],
                            wh3[:, g],
                            start=(i == 0),
                            stop=False,
                        )

            # --- final projection: Y = (-h)^T @ (-W_ho^T) + bias ---
            if not fillers:
                psy = fpool.tile([BC, O], f32, tag="fy", name="psy")
                wh3 = whot[:].rearrange("p (g o) -> p g o", g=NCH)
            tail = [g for g in range(NCH) if g not in fillers]
            for i, g in enumerate(tail):
                nc.tensor.matmul(
                    psy[:],
                    hbuf[:, ds(g * BC, BC)],
                    wh3[:, g],
                    start=(not fillers) and i == 0,
                    stop=(i == len(tail) - 1),
                )
            y_t = ypool.tile([BC, O], f32, tag="y", name="y_t")
            nc.vector.tensor_tensor(y_t[:], psy[:], bias_t[:], mybir.AluOpType.add)
            nc.sync.dma_start(Y, y_t[:])
    nc.compile()
    return nc


def _get_program(plan):
    key = (plan["first_block"], plan["fwidth"], os.environ.get("DIAG_GP"))
    if key not in _CACHE:
        _CACHE[key] = _build(
            plan["first_block"],
            plan["fwidth"],
            plan["sclen"],
            plan["nearly"],
            plan["seg_off"],
        )
    return _CACHE[key]


def _ensure_ntff_hook():
    """Provide antenv.axon_hooks (absent in this image) so trace=True works."""
    import sys
    import types

    if "antenv.axon_hooks" in sys.modules:
        return True
    try:
        import antenv

        mod = types.ModuleType("antenv.axon_hooks")
        mod._hook = None

        def set_axon_ntff_profile_hook(h):
            mod._hook = h

        def get_axon_ntff_profile_hook():
            return mod._hook

        mod.set_axon_ntff_profile_hook = set_axon_ntff_profile_hook
        mod.get_axon_ntff_profile_hook = get_axon_ntff_profile_hook
        sys.modules["antenv.axon_hooks"] = mod
        antenv.axon_hooks = mod

        from trn_agent_boot.trn_boot import _ntff_profile_via_ctypes

        hook = _ntff_profile_via_ctypes("/opt/axon/libaxon_pjrt.so")
        mod.set_axon_ntff_profile_hook(hook)
        return hook is not None
    except Exception:
        return False


def kernel(X, W_ih, hh, W_ho, b_ho):
    import ml_dtypes
    from concourse import bass_utils

    bf16 = ml_dtypes.bfloat16
    X = np.asarray(X, dtype=np.float32)
    W_ih = np.asarray(W_ih, dtype=np.float32)
    hh = np.asarray(hh, dtype=np.float32)
    W_ho = np.asarray(W_ho, dtype=np.float32)
    b_ho = np.asarray(b_ho, dtype=np.float32)

    plan = _make_plan(hh)
    perm = plan["perm"]
    nc = _get_program(plan)

    wiht = np.ascontiguousarray(W_ih[perm].T).astype(bf16)  # [I, H]
    whot = np.ascontiguousarray(-W_ho[:, perm].T).astype(bf16)  # [H, O], negated
    bias = np.tile(b_ho[None, :], (BC, 1)).astype(np.float32)

    common = {
        "WIHT": wiht,
        "WHOT": whot,
        "BIAS": bias,
        "SC": plan["SC"].astype(bf16),
    }
    in_maps = []
    for m in range(NCORES):
        im = dict(common)
        xm = X[:, m * BC : (m + 1) * BC, :]  # [S, BC, I]
        # device layout [NBLK/2, 128(i-within), (block-in-pair, ic, b, tau)]
        xt = xm.reshape(NBLK // 2, 2, TB, BC, NI, 128).transpose(0, 5, 1, 4, 3, 2)
        im["X"] = (
            np.ascontiguousarray(xt)
            .reshape(NBLK // 2, 128, 2 * NI * BC * TB)
            .astype(bf16)
        )
        in_maps.append(im)

    trace = bool(int(os.environ.get("DIAG_TRACE", "0")))
    if trace:
        trace = _ensure_ntff_hook()
    res = None
    for attempt in range(3):
        try:
            res = bass_utils.run_bass_kernel_spmd(
                nc,
                in_maps,
                core_ids=list(range(NCORES)),
                trace=trace,
                tmpdir=os.environ.get("DIAG_TRACE_DIR") or None,
            )
            break
        except Exception:
            if attempt == 2:
                raise
            trace = False  # retry without profiling
    if res.exec_time_ns is not None:
        kernel.last_exec_time_ns = res.exec_time_ns
        kernel.last_mean_exec_time_ns = res.mean_exec_time_ns
    Yfull = np.concatenate([r["Y"] for r in res.results], axis=0)
    return Yfull


kernel.last_exec_time_ns = None
kernel.last_mean_exec_time_ns = None


# revision 57
# speedup vs baseline: 1.0317x; 1.0159x over previous
"""Trainium2 Bass kernel for nn_Diagnet (S=1024, B=64, I=512, H=2048, O=512).

    u = einsum('sbi,hi->sbh', X, W_ih)
    h_t = |u_t + hh * h_{t-1}|   (scan over S, only final h needed)
    Y = h_final @ W_ho.T + b_ho

Strategy (8 NeuronCores, data-parallel over batch, 8 batch rows per core):

* H lanes are permuted so hh is sorted descending and split into 16
  chunks of 128.  With per-lane decay a = hh < 1, a chunk whose largest
  a satisfies a^K < 1e-10 only needs the last K steps (exact to ~1e-10,
  far below fp32 noise): the GEMM and the scan skip everything earlier.
* The scan runs as ONE custom DVE instruction per (chunk, 64-step
  block): a hand-tuned micro-op program (SCANDIAG_ANT) folds
  m_k = |m_{k-1} - x_k * sc_k| along the free dimension at 1 elem/cycle,
  with the 8 batch chains as sub-dimension pages that re-init the fold
  state to 0 at each page boundary.  State is kept pre-scaled
  (m = a^(K-1-t) h, scales folded into the sc stream), so the step is
  multiply-free in the recurrence itself and no inter-block rescale is
  needed.  Block-to-block state is chained by a carry element prepended
  to each page, written by the previous block's scan through a stride-0
  output AP (65 writes to one address; last write = final state).  The
  scan outputs -m so the carry re-seeds exactly: |0 - (-m)| = m.
* GEMM in bf16 (1 PE cycle/row vs 4 for fp32): X and W_ih^T are cast
  host-side.  PSUM drains to SBUF as plain copies on the otherwise-idle
  Activation engine; the scale (and its sign flip) ride the scan's
  second input stream.  A chunk's first active block has no carry, so
  its scan reads PSUM directly (no copy, no memset) and covers only the
  chunk's true truncation tail (unrounded window).
* Scheduling: chunks whose whole window is the last block have zero
  dependency on the rest of the scan; they run as fillers during the
  chunk-0-only phase where the engines idle on X DMAs.  X streams in
  2-block-pair DMAs (DMA-completion semaphores cost ~2us each), and
  large constants are issued at points chosen to keep them out of the
  phase-1 HBM burst.
* Final projection: h arrives in [128, (chunk, batch)] layout; 16
  accumulating bf16 matmuls against host-transposed-negated W_ho^T
  (negated because the scan hands over -h), bias added on DVE.  Filler
  chunks' matmuls run early, off the tail.
"""

import math
import os

from contextlib import ExitStack

import numpy as np

S, B, I, H, O = 1024, 64, 512, 2048, 512
NCORES = 8
BC = B // NCORES  # 8 batch rows per core
TB = 64  # time block
NBLK = S // TB  # 16
NCH = H // 128  # 16 h-chunks
NI = I // 128  # 4 i-chunks
NSLOT = TB + 1  # carry + 64 scan elements per page
LN_TRUNC = 23.03  # a^K <= e^-23 ~ 1e-10 -> truncate

_CACHE = {}


def _register_scandiag():
    """Custom DVE op: per page s of [P, S, N] in0 (in1 broadcast over pages):
        m = 0
        for k: m = |m - in0[p,s,k] * in1[p,k]| ; out[p,s,k] = -m
    Built from lower() of a Spec, with the step uop hand-edited so the
    fold state re-inits to 0 at each page boundary."""
    import concourse.dve_ops as dve_ops
    from concourse.dve_spec import Spec, Src0, Src1, Zero, scan, PageIdx, lower, AluOp
    from concourse.dve_uop import DveOpSpec, AluInp

    name = "SCANDIAG_ANT"
    for op in dve_ops.OPS:
        if op.name == name:
            return op

    body = (Zero - scan(AluOp.ABSOLUTE_DIFF, Src0 * Src1, init=Zero)) + PageIdx(
        Zero, Zero
    )

    def ref(in0, in1, s0, s1, imm2):
        x = in0.astype(np.float32)
        w = in1.astype(np.float32)
        if x.ndim == 2:
            x = x[:, None, :]
        if w.ndim == 3:
            w = w[:, 0, :]
        out = np.empty_like(x)
        for s in range(x.shape[1]):
            m = np.zeros(x.shape[0], dtype=np.float32)
            for k in range(x.shape[2]):
                m = np.abs(m - x[:, s, k] * w[:, k])
                out[:, s, k] = -m
        return out.reshape(in0.shape)

    spec = Spec(body=body, reference=ref)
    row = max(dve_ops._SUB_OPCODE_FOR_NAME.values()) + 1
    assert row < 0x20
    shas = {}
    compiled = {}
    for ver in ("v3", "v4"):
        uops = lower(spec, ver=ver)
        assert len(uops) == 3, (ver, len(uops))
        hits = [
            k
            for k, b in enumerate(uops[2].datapath_config)
            if b.op == AluOp.ABSOLUTE_DIFF and b.alu_src0 == AluInp.CURR_ALU_OUT
        ]
        assert len(hits) == 1, hits
        k = hits[0]
        seed_blk = uops[0].datapath_config[k]
        assert seed_blk.op == AluOp.BYPASS
        uops[2].datapath_config[k].alu_src0 = seed_blk.alu_src0
        s = DveOpSpec(name=name, opcode=row, uops=uops, rd1_en=True)
        shas[ver] = s.sha(ver)
        compiled[ver] = s
    op = dve_ops.DveOp(name, spec, subdim=True, uops_sha=shas)
    for ver in ("v3", "v4"):
        dve_ops._COMPILE_CACHE[(name, ver)] = compiled[ver]
    dve_ops._SUB_OPCODE_FOR_NAME[name] = row
    dve_ops.OPS.append(op)
    dve_ops.CUSTOM_DVE_SPECS[name] = spec
    return op


def _make_plan(hh):
    a = np.maximum(np.abs(hh.astype(np.float64)), 1e-30)
    perm = np.argsort(-a, kind="stable")
    a_s = a[perm]
    first_block = []
    fwidth = []  # steps computed in the chunk's first block (1..TB)
    for g in range(NCH):
        amax = a_s[g * 128]
        if amax >= math.exp(-LN_TRUNC / S):
            kg = S
        else:
            kg = min(S, max(1, int(math.ceil(LN_TRUNC / math.log(1.0 / amax)))))
        nblk_g = (kg + TB - 1) // TB
        first_block.append(NBLK - nblk_g)
        fwidth.append(kg - (nblk_g - 1) * TB)
    assert all(
        first_block[g] <= first_block[g + 1] for g in range(NCH - 1)
    ), first_block

    # SC stream: kb-major variable-width segments. A chunk's first block
    # covers only its true window tail (width fwidth[g], no carry slot);
    # later blocks are 65 wide: carry slot (1.0) + 64 scales.
    # scale slot for step tau-from-window-start: -a^(K-1-tau).
    ag = a_s.reshape(NCH, 128).T  # [128, NCH]
    segs = [(g, kb) for kb in range(NBLK) for g in range(NCH) if first_block[g] <= kb]
    seg_off = {}
    widths = {}
    off = 0
    for g, kb in segs:
        w = fwidth[g] if kb == first_block[g] else NSLOT
        seg_off[(g, kb)] = off
        widths[(g, kb)] = w
        off += w
    sc = np.zeros((128, off), dtype=np.float64)
    for (g, kb), o in seg_off.items():
        kg = fwidth[g] + (NBLK - 1 - first_block[g]) * TB
        if kb == first_block[g]:
            w = fwidth[g]
            tau = np.arange(w)
            e = kg - 1 - tau
            sc[:, o : o + w] = -(ag[:, g : g + 1] ** e[None, :])
        else:
            tau = np.arange(TB)
            e = kg - 1 - (fwidth[g] + (kb - 1 - first_block[g]) * TB + tau)
            sc[:, o] = 1.0
            sc[:, o + 1 : o + NSLOT] = -(ag[:, g : g + 1] ** e[None, :])
    nearly = sum(widths[s] for s in segs if s[1] < first_block[1])
    return {
        "perm": perm,
        "first_block": tuple(first_block),
        "fwidth": tuple(fwidth),
        "SC": sc,  # float64; cast at upload
        "sclen": off,
        "nearly": max(nearly, 1),
        "seg_off": seg_off,
    }


def _build(first_block, fwidth, sclen, nearly, seg_off):
    import concourse.mybir as mybir
    import concourse.tile as tile
    from concourse import bacc
    from concourse.bass import ds

    SCANDIAG = _register_scandiag()
    f32 = mybir.dt.float32
    bf16 = mybir.dt.bfloat16

    nc = bacc.Bacc("TRN2", target_bir_lowering=False, debug=False, num_devices=NCORES)
    NPAIR = NBLK // 2
    BLKF = NI * BC * TB  # free elems per block
    X = nc.dram_tensor("X", [NPAIR, 128, 2 * BLKF], bf16, kind="ExternalInput").ap()
    WIHT = nc.dram_tensor("WIHT", [I, H], bf16, kind="ExternalInput").ap()
    SC = nc.dram_tensor("SC", [128, sclen], bf16, kind="ExternalInput").ap()
    WHOT = nc.dram_tensor("WHOT", [H, O], bf16, kind="ExternalInput").ap()
    BIAS = nc.dram_tensor("BIAS", [BC, O], f32, kind="ExternalInput").ap()
    Y = nc.dram_tensor("Y", [BC, O], f32, kind="ExternalOutput").ap()

    acts = [[g for g in range(NCH) if first_block[g] <= kb] for kb in range(NBLK)]

    with tile.TileContext(nc) as tc:
        with ExitStack() as ctx:
            consts = ctx.enter_context(tc.tile_pool(name="consts", bufs=1))
            xtpool = ctx.enter_context(
                tc.tile_pool(name="xt", bufs=int(os.environ.get("DIAG_XB", "3")))
            )
            upool = ctx.enter_context(tc.tile_pool(name="ubuf", bufs=int(os.environ.get("DIAG_UB", "3"))))
            ypool = ctx.enter_context(tc.tile_pool(name="yout", bufs=1))
            gpool = ctx.enter_context(
                tc.tile_pool(name="gpsum", bufs=int(os.environ.get("DIAG_GP", "7")), space="PSUM")
            )
            fpool = ctx.enter_context(tc.tile_pool(name="fpsum", bufs=1, space="PSUM"))

            # --- constants ---
            # block 0's first half goes out as the very first Sync-queue DMA
            # (its sibling half issues on GpSimd in parallel) so the two
            # critical head transfers don't serialize on one sequencer.
            xpair0 = xtpool.tile([128, 2 * NI * BC * TB], bf16, tag="xt", name="xt_0")
            nc.sync.dma_start(xpair0[:, 0 : NI * BC * TB], X[0][:, 0 : NI * BC * TB])
            # chunk-0 columns as a small separate tile: blocks 0..fb[1]-1 only
            # touch chunk 0, and this 128KB DMA unblocks the first GEMMs fast.
            wih0 = consts.tile([128, NI * 128], bf16, tag="wih0", name="wih0")
            nc.sync.dma_start(
                wih0[:].rearrange("p (i h) -> p i h", i=NI),
                WIHT.rearrange("(i p) h -> p i h", p=128)[:, :, 0:128],
            )
            sc_t = consts.tile([128, sclen], bf16, tag="sc", name="sc_t")
            ne = nearly
            nc.sync.dma_start(sc_t[:, 0:ne], SC[:, 0:ne])
            wiht = consts.tile([128, NI * H], bf16, tag="wiht", name="wiht")
            w4 = wiht[:].rearrange("p (i h) -> p i h", i=NI)
            hbuf = consts.tile([128, NCH * BC], bf16, tag="hbuf", name="hbuf")

            ucur = {}

            def wslice(g, ic):
                if g == 0:
                    return wih0[:].rearrange("p (i h) -> p i h", i=NI)[:, ic]
                return w4[:, ic, ds(g * 128, 128)]

            def process_seg(g, kb, x3):
                first = kb == first_block[g]
                last = kb == NBLK - 1
                off = seg_off[(g, kb)]
                w = fwidth[g] if first else TB
                ps = gpool.tile([128, w * BC], f32, tag="gp", name=f"gp_{kb}_{g}")
                ps3 = ps[:].rearrange("p (s n) -> p s n", s=BC)
                x4 = x3.rearrange("p i (s t) -> p i s t", s=BC)
                for ic in range(NI):
                    nc.tensor.matmul(
                        ps3,
                        wslice(g, ic),
                        x4[:, ic, :, TB - w : TB],
                        start=(ic == 0),
                        stop=(ic == NI - 1),
                    )
                if first:
                    # no carry yet: scan straight out of PSUM, skip the copy;
                    # only the chunk's true window tail (w steps) is computed
                    in0 = ps3
                    width = w
                    scs = sc_t[:, ds(off, w)].unsqueeze(1).broadcast_to(
                        [128, BC, w]
                    )
                else:
                    u3 = ucur[g][:].rearrange("p (s n) -> p s n", n=NSLOT)
                    nc.scalar.copy(u3[:, :, 1:NSLOT], ps3)
                    in0 = u3
                    width = NSLOT
                    scs = sc_t[:, ds(off, NSLOT)].unsqueeze(1).broadcast_to(
                        [128, BC, NSLOT]
                    )
                if last:
                    out_ap = hbuf[:, ds(g * BC, BC)].broadcast_to([128, BC, width])
                else:
                    u_nxt = upool.tile(
                        [128, BC * NSLOT], f32, tag=f"u{g}", name=f"u_{g}_{kb + 1}"
                    )
                    out_ap = (
                        u_nxt[:]
                        .rearrange("p (s n) -> p s n", n=NSLOT)[:, :, 0:1]
                        .broadcast_to([128, BC, width])
                    )
                nc.vector._custom_dve(SCANDIAG, out=out_ap, in0=in0, in1=scs)
                if not last:
                    ucur[g] = u_nxt

            # chunks whose whole window is block 15: zero deps on the chain.
            # Run them as fillers during the chunk-0-only phase, where DVE/Act
            # and the PE are mostly idle waiting on X DMAs.
            if int(os.environ.get("DIAG_FILL", "1")):
                fillers = [g for g in range(1, NCH) if first_block[g] == NBLK - 1]
            else:
                fillers = []
            FILL_AT = list(range(8, 14))
            fill_sched = {kb: [] for kb in FILL_AT}
            for i, g in enumerate(fillers):
                fill_sched[FILL_AT[i % len(FILL_AT)]].append(g)

            x15 = consts.tile([128, 2 * BLKF], bf16, tag="x15", name="x15")
            x15v = x15[:].rearrange("p (two i n) -> p two i n", two=2, i=NI)

            xpair = None
            for kb in range(NBLK):
                # X: one DMA per 2-block pair (each DMA completion semaphore
                # costs ~2us, so fewer/larger transfers win); block 0 gets its
                # own half-DMA so the pipeline starts sooner; the last pair
                # lives in a dedicated tile loaded early for the fillers.
                if kb % 2 == 0 and kb < NBLK - 2:
                    if kb == 0:
                        xpair = xpair0  # first half already in flight on Sync
                        nc.gpsimd.dma_start(xpair[:, BLKF : 2 * BLKF], X[0][:, BLKF:])
                    else:
                        xpair = xtpool.tile(
                            [128, 2 * BLKF], bf16, tag="xt", name=f"xt_{kb}"
                        )
                        nc.gpsimd.dma_start(xpair[:], X[kb // 2])
                if kb < NBLK - 2:
                    x3 = xpair[:].rearrange(
                        "p (two i n) -> p two i n", two=2, i=NI
                    )[:, kb % 2]
                else:
                    x3 = x15v[:, kb % 2]

                if kb == 2:
                    # rest of the scale table (not needed until block fb[1])
                    nc.sync.dma_start(sc_t[:, ne:], SC[:, ne:])
                if kb == 3:
                    # last block pair, needed by fillers from kb~8
                    nc.gpsimd.dma_start(x15[:], X[NPAIR - 1])
                if kb == 5:
                    # full W_ih columns: first needed by the fillers
                    nc.sync.dma_start(w4, WIHT.rearrange("(i p) h -> p i h", p=128))
                if kb == 8:
                    # needed only at the end; issued here to overlap
                    whot = consts.tile([128, NCH * O], bf16, tag="whot", name="whot")
                    nc.sync.dma_start(
                        whot[:].rearrange("p (g o) -> p g o", g=NCH),
                        WHOT.rearrange("(g p) o -> p g o", p=128),
                    )
                    bias_t = ypool.tile([BC, O], f32, tag="bias", name="bias_t")
                    nc.sync.dma_start(bias_t[:], BIAS)

                for g in acts[kb]:
                    if kb == NBLK - 1 and g in fillers:
                        continue  # already done as a filler
                    process_seg(g, kb, x3)
                for g in fill_sched.get(kb, ()):
                    process_seg(g, NBLK - 1, x15v[:, 1])

                if kb == 13 and fillers:
                    # final-projection matmuls for the filler chunks: their h
                    # is final and W_ho has landed, so take them off the tail
                    psy = fpool.tile([BC, O], f32, tag="fy", name="psy")
                    wh3 = whot[:].rearrange("p (g o) -> p g o", g=NCH)
                    for i, g in enumerate(fillers):
                        nc.tensor.matmul(
                            psy[:],
                            hbuf[:, ds(g * BC, BC)],
                            wh3[:, g],
                            start=(i == 0),
                            stop=False,
                        )

            # --- final projection: Y = (-h)^T @ (-W_ho^T) + bias ---
            if not fillers:
                psy = fpool.tile([BC, O], f32, tag="fy", name="psy")
                wh3 = whot[:].rearrange("p (g o) -> p g o", g=NCH)
            tail = [g for g in range(NCH) if g not in fillers]
            for i, g in enumerate(tail):
                nc.tensor.matmul(
                    psy[:],
                    hbuf[:, ds(g * BC, BC)],
                    wh3[:, g],
                    start=(not fillers) and i == 0,
                    stop=(i == len(tail) - 1),
                )
            y_t = ypool.tile([BC, O], f32, tag="y", name="y_t")
            nc.vector.tensor_tensor(y_t[:], psy[:], bias_t[:], mybir.AluOpType.add)
            nc.sync.dma_start(Y, y_t[:])
    nc.compile()
    return nc


def _get_program(plan):
    key = (plan["first_block"], plan["fwidth"], os.environ.get("DIAG_GP"))
    if key not in _CACHE:
        _CACHE[key] = _build(
            plan["first_block"],
            plan["fwidth"],
            plan["sclen"],
            plan["nearly"],
            plan["seg_off"],
        )
    return _CACHE[key]


def _ensure_ntff_hook():
    """Provide antenv.axon_hooks (absent in this image) so trace=True works."""
    import sys
    import types

    if "antenv.axon_hooks" in sys.modules:
        return True
    try:
        import antenv

        mod = types.ModuleType("antenv.axon_hooks")
        mod._hook = None

        def set_axon_ntff_profile_hook(h):
            mod._hook = h

        def get_axon_ntff_profile_hook():
            return mod._hook

        mod.set_axon_ntff_profile_hook = set_axon_ntff_profile_hook
        mod.get_axon_ntff_profile_hook = get_axon_ntff_profile_hook
        sys.modules["antenv.axon_hooks"] = mod
        antenv.axon_hooks = mod

        from trn_agent_boot.trn_boot import _ntff_profile_via_ctypes

        hook = _ntff_profile_via_ctypes("/opt/axon/libaxon_pjrt.so")
        mod.set_axon_ntff_profile_hook(hook)
        return hook is not None
    except Exception:
        return False


def kernel(X, W_ih, hh, W_ho, b_ho):
    import ml_dtypes
    from concourse import bass_utils

    bf16 = ml_dtypes.bfloat16
    X = np.asarray(X, dtype=np.float32)
    W_ih = np.asarray(W_ih, dtype=np.float32)
    hh = np.asarray(hh, dtype=np.float32)
    W_ho = np.asarray(W_ho, dtype=np.float32)
    b_ho = np.asarray(b_ho, dtype=np.float32)

    plan = _make_plan(hh)
    perm = plan["perm"]
    nc = _get_program(plan)

    wiht = np.ascontiguousarray(W_ih[perm].T).astype(bf16)  # [I, H]
    whot = np.ascontiguousarray(-W_ho[:, perm].T).astype(bf16)  # [H, O], negated
    bias = np.tile(b_ho[None, :], (BC, 1)).astype(np.float32)

    common = {
        "WIHT": wiht,
        "WHOT": whot,
        "BIAS": bias,
        "SC": plan["SC"].astype(bf16),
    }
    in_maps = []
    for m in range(NCORES):
        im = dict(common)
        xm = X[:, m * BC : (m + 1) * BC, :]  # [S, BC, I]
        # device layout [NBLK/2, 128(i-within), (block-in-pair, ic, b, tau)]
        xt = xm.reshape(NBLK // 2, 2, TB, BC, NI, 128).transpose(0, 5, 1, 4, 3, 2)
        im["X"] = (
            np.ascontiguousarray(xt)
            .reshape(NBLK // 2, 128, 2 * NI * BC * TB)
            .astype(bf16)
        )
        in_maps.append(im)

    trace = bool(int(os.environ.get("DIAG_TRACE", "0")))
    if trace:
        trace = _ensure_ntff_hook()
    res = None
    for attempt in range(3):
        try:
            res = bass_utils.run_bass_kernel_spmd(
                nc,
                in_maps,
                core_ids=list(range(NCORES)),
                trace=trace,
                tmpdir=os.environ.get("DIAG_TRACE_DIR") or None,
            )
            break
        except Exception:
            if attempt == 2:
                raise
            trace = False  # retry without profiling
    if res.exec_time_ns is not None:
        kernel.last_exec_time_ns = res.exec_time_ns
        kernel.last_mean_exec_time_ns = res.mean_exec_time_ns
    Yfull = np.concatenate([r["Y"] for r in res.results], axis=0)
    return Yfull


kernel.last_exec_time_ns = None
kernel.last_mean_exec_time_ns = None
